# revision 1
# baseline (speedup 1.0000x reference)
"""Causal MHA (B=1, S=4096, 16 heads x 64, hidden 1024) on 8 TRN2 cores — v2.

Sharding: tensor-parallel over heads, 2 heads/core (per the sharding hint);
each core writes a full-shape fp16 partial of the output projection and the
host sums the 8 partials (the TP all-reduce).

v2 vs the previous kernel — built from a hardware-calibrated cost model
(microbenched on this device):
  - PE matmul cost is rows x 0.42ns @2.4GHz ONLY while the PE stays
    continuously busy; gaps reset the DVFS ramp to 1.2GHz (3us to recover).
    Emission is driven by a static scheduler tracking per-engine clocks so
    the PE queue never exposes an unsatisfied wait.
  - PV is computed TRANSPOSED: OT[sq,65] += pt[sk,sq-chunk]^T @ (v|1)[sk,:],
    pt stationary. Moving width drops 512 -> 65 with full 128x128 MACs/row
    (~2x fewer PE cycles); N=65 accumulating matmuls sustain 30ns on HW.
  - v-projection emits [s,e] directly (x stationary, wv moving): no flip
    transposes, one fused eviction pass.
  - softmax denominator from the ones-column of (v|1); normalization is a
    per-partition tensor_scalar multiply on OT eviction (l sits on the sq
    partition after the transposed PV) — no broadcast matmuls, no Ln/Exp.
  - fp16 output partials (host all-reduce in fp32) halve store traffic.
  - PSUM start=True zeroes a whole 2KB bank: multi-chain banks use
    start=True only on the bank's first matmul (rest accumulate onto the
    bank's pending-zero bytes).
  - warm-up matmuls during the initial DMA fill ramp the PE clock.
"""
import sys
sys.path.insert(0, "/opt/trn_rl_repo")

import numpy as np

import concourse.bass as bass
import concourse.mybir as mybir
import concourse.tile as tile
from concourse.bass_utils import run_bass_kernel_spmd

# ---------------------------------------------------------------- constants
S = 4096
HID = 1024
NCORES = 8
HPC = 2            # heads per core
HD = 64
EPC = HPC * HD     # 128
SB = 512           # q-block width
NB = S // SB       # 8
NT = S // 128      # 32 k-tiles
KH = HID // 128    # 8 contraction chunks
GROUP = 2          # k-tiles per exp batch

F32 = mybir.dt.float32
F16 = mybir.dt.float16
AF = mybir.ActivationFunctionType

_MAX_WAITS = 1

# calibrated cost model (ns)
PE_NS = 0.43       # per moving row at full clock
PE_OV = 3.0        # per-instruction overhead
SC_NS = 0.95       # scalar activation per column (under load)
SC_OV = 160.0      # per-activation overhead
DV_PS = 0.85       # DVE per column (psum-involved f32)
DV_OV = 270.0      # DVE per-op overhead (psum access)
SEM = 110.0        # semaphore propagation


def _split_waits(nc):
    """Hoist extra sync-waits onto inserted same-engine wait carriers
    (this walrus build allows a single sync-wait per instruction)."""
    n = 0
    for fn in nc.m.functions:
        for bb in fn.blocks:
            insts = bb.instructions
            i = 0
            while i < len(insts):
                inst = insts[i]
                si = inst.sync_info
                w = list(si.on_wait) if si is not None and si.on_wait else []
                if len(w) > _MAX_WAITS:
                    chunks = [w[j:j + _MAX_WAITS] for j in range(0, len(w), _MAX_WAITS)]
                    si.on_wait = chunks[-1]
                    for ch in chunks[:-1]:
                        d = mybir.InstEventSemaphore(
                            name=f"{inst.name}_ws{n}", ins=[], outs=[])
                        d.engine = inst.engine
                        d.sync_info = mybir.SyncInfo(on_wait=ch, on_update=[])
                        insts.insert(i, d)
                        i += 1
                        n += 1
                i += 1
    return n


class Unit:
    __slots__ = ("ready", "emit", "blk", "cost", "phase")

    def __init__(self, ready, emit, blk=-1, cost=300.0, phase="pre"):
        self.ready = ready
        self.emit = emit
        self.blk = blk
        self.cost = cost
        self.phase = phase


def _build_nc():
    nc = bass.Bass(target_bir_lowering=False)

    xT = nc.declare_dram_parameter("xT", [NB, 128, KH * SB], F16, isOutput=False)
    wqT = nc.declare_dram_parameter("wqT", [128, KH * EPC], F16, isOutput=False)
    wkT = nc.declare_dram_parameter("wkT", [128, KH * EPC], F16, isOutput=False)
    wvT = nc.declare_dram_parameter("wvT", [128, KH * EPC], F16, isOutput=False)
    woT = nc.declare_dram_parameter("woT", [EPC, HID], F16, isOutput=False)
    out = nc.declare_dram_parameter("out", [S, HID], F16, isOutput=True)

    with tile.TileContext(nc) as tc:
        with tc.tile_pool(name="const", bufs=1) as const, \
             tc.tile_pool(name="qk", bufs=1) as qk, \
             tc.tile_pool(name="xt", bufs=NB) as xtp, \
             tc.tile_pool(name="pt", bufs=4) as ptp, \
             tc.tile_pool(name="att", bufs=2) as attp, \
             tc.tile_pool(name="atts", bufs=2) as attsp, \
             tc.tile_pool(name="osb", bufs=8) as osbp, \
             tc.tile_pool(name="rc", bufs=4) as rcp, \
             tc.tile_pool(name="st", bufs=2, space="PSUM") as stp, \
             tc.tile_pool(name="ot", bufs=2, space="PSUM") as otp, \
             tc.tile_pool(name="dr", bufs=2, space="PSUM") as drp:

            # ---------------- SBUF tiles
            wq_sb = const.tile([128, KH, EPC], F16, tag="wq")
            wk_sb = const.tile([128, KH, EPC], F16, tag="wk")
            wv_sb = const.tile([128, KH, EPC], F16, tag="wv")
            wo_sb = const.tile([EPC, HID], F16, tag="wo")
            id_sb = const.tile([128, 128], F16, tag="id")
            warm = const.tile([128, 512], F16, tag="warm")
            qT = qk.tile([128, S], F16, tag="qT")
            kT = qk.tile([128, S], F16, tag="kT")
            vbuf = qk.tile([128, HPC, NT, 65], F16, tag="v")

            # ---------------- engine clocks (ns, est.) for static scheduling
            clk = {"pe": 0.0, "sc": 0.0, "dv": 0.0}

            def pe(rows, n=1):
                clk["pe"] += rows * PE_NS + n * PE_OV
                return clk["pe"]

            def sc(cols, dep=0.0):
                clk["sc"] = max(clk["sc"], dep + SEM, clk["sc"]) + cols * SC_NS + SC_OV
                return clk["sc"]

            def dv(cols, dep=0.0, per=DV_PS, ov=DV_OV):
                clk["dv"] = max(clk["dv"], dep + SEM) + cols * per + ov
                return clk["dv"]

            # ---------------- initial DMAs
            nc.vector.memset(warm, 0.125)
            # identity matrix generated on-chip: ones -> keep only the
            # diagonal (i - p == 0). No DMA -> no collapsed ring-sem wait.
            nc.vector.memset(id_sb, 1.0)
            nc.gpsimd.affine_select(
                out=id_sb, in_=id_sb, pattern=[[1, 128]],
                compare_op=mybir.AluOpType.is_equal, fill=0.0,
                channel_multiplier=-1)
            # only the ones-column of (v|1) needs init; v-proj evictions
            # write columns 0:64 before any PV reads them
            nc.vector.memset(vbuf[:, :, :, 64:65].rearrange(
                "p a b c -> p (a b c)"), 1.0)

            xts = {}

            def load_xt(b, split=False):
                # host pre-packs x as [b][p][k*s]: 128 contiguous 8KB rows
                # per block-tile (vs 1024 x 1KB strided descriptors)
                xt = xtp.tile([128, KH, SB], F16, tag="xt", name=f"xt{b}")
                dst = xt.rearrange("p k s -> p (k s)")
                half = KH * SB // 2
                if split:
                    nc.sync.dma_start(out=dst[:, 0:half], in_=xT[b, :, 0:half])
                    nc.sync.dma_start(out=dst[:, half:], in_=xT[b, :, half:])
                else:
                    nc.sync.dma_start(out=dst, in_=xT[b, :, :])
                xts[b] = xt

            # DMA emission is interleaved with compute emission below: the
            # tile framework collapses DMA waits into a ring-counter wait, so
            # each consumer must be emitted before unrelated loads are queued
            nc.sync.dma_start(out=wq_sb.rearrange("p k m -> p (k m)"), in_=wqT[:, :])
            load_xt(0, split=True)

            # warm-up: ramp the PE DVFS while the x/weight DMAs land
            for r in range(15):
                wps = drp.tile([128, 512], F32, tag="dr", name=f"warm{r}")
                nc.tensor.matmul(wps, warm[:, 0:128], warm,
                                 start=True, stop=True)
            clk["pe"] = 6000.0   # DMA-gated start + ramp span

            # ---------------- work units
            def u_qk_proj(b, which):
                w_sb, dst = (wq_sb, qT) if which == "q" else (wk_sb, kT)

                def emit():
                    ps = drp.tile([128, SB], F32, tag="dr", name=f"p{which}{b}")
                    for k in range(KH):
                        nc.tensor.matmul(ps, w_sb[:, k, :], xts[b][:, k, :],
                                         start=(k == 0), stop=(k == KH - 1))
                    t = pe(KH * SB, KH)
                    nc.vector.tensor_copy(
                        out=dst[:, b * SB:(b + 1) * SB], in_=ps)
                    dv(SB, dep=t)
                return emit

            def u_v_proj(b):
                def emit():
                    vps = drp.tile([128, 4, 128], F32, tag="dr", name=f"pv{b}")
                    for k in range(KH):
                        for c in range(4):
                            nc.tensor.matmul(
                                vps[:, c, :], xts[b][:, k, c * 128:(c + 1) * 128],
                                wv_sb[:, k, :],
                                start=(k == 0 and c == 0), stop=(k == KH - 1),
                                skip_group_check=True)
                    t = pe(KH * 4 * 128, KH * 4)
                    src = vps.rearrange("p c (h d) -> p h c d", h=HPC)
                    nc.vector.tensor_copy(
                        out=vbuf[:, :, 4 * b:4 * b + 4, 0:64], in_=src)
                    dv(512, dep=t)
                return emit

            epi = []     # ready-gated epilogue units
            filler = []  # proj units, tagged by the block whose slots need them

            PCOST = KH * SB * PE_NS

            def push_proj(b):
                filler.append(Unit(lambda: 0.0, u_qk_proj(b, "q"), blk=b,
                                   cost=PCOST, phase="pre"))
                filler.append(Unit(lambda: 0.0, u_qk_proj(b, "k"), blk=b,
                                   cost=PCOST, phase="mid"))
                filler.append(Unit(lambda: 0.0, u_v_proj(b), blk=b,
                                   cost=PCOST, phase="mid"))

            cur_blk = [0]

            def pick_unit(limit):
                if filler and filler[0].blk <= cur_blk[0] + 1 \
                        and filler[0].cost <= limit:
                    return filler.pop(0)
                for i, u in enumerate(epi):
                    if u.ready() <= clk["pe"] and u.cost <= limit:
                        return epi.pop(i)
                if filler and filler[0].cost <= limit:
                    return filler.pop(0)
                return None

            def drip_one():
                u = pick_unit(1e18)
                if u is None and epi:
                    u = epi.pop(0)
                if u is None:
                    return False
                u.emit()
                return True

            def drip_until(t):
                while clk["pe"] < t - 60.0:
                    u = pick_unit(t + 350.0 - clk["pe"])
                    if u is None:
                        clk["pe"] = t
                        break
                    u.emit()

            # ---------------- per-half epilogue (chunks {0,1} / {2,3})
            epi_state = {}

            def make_epi(b):
                att = attp.tile([128, 4, HPC, 64], F16, tag="att", name=f"att{b}")
                attTs = attsp.tile([128, 4, 128], F16, tag="attTs", name=f"aT{b}")
                rc = rcp.tile([128, HPC, 4], F32, tag="rc", name=f"rc{b}")
                st8 = {"nd": [1e18, 1e18], "ev": {}, "osb": {}}
                epi_state[b] = (att, attTs, rc, st8)
                return epi_state[b]

            def norm_half(b, half, ot_t):
                att, attTs, rc, st8 = epi_state[b]
                ots = ot_tiles[b]
                cs = (0, 1) if half == 0 else (2, 3)

                def emit():
                    for h in range(HPC):
                        nc.vector.reciprocal(
                            out=rc[:, h, cs[0]:cs[1] + 1],
                            in_=ots[h][:, cs[0]:cs[1] + 1, 64:65].rearrange(
                                "p c o -> p (c o)"))
                        dv(2, dep=ot_t["t"])
                    for h in range(HPC):
                        for c in cs:
                            nc.vector.tensor_scalar_mul(
                                att[:, c, h, :], ots[h][:, c, 0:64],
                                rc[:, h, c:c + 1])
                            st8["nd"][half] = dv(64)
                return emit

            def push_tr_op(b, half):
                att, attTs, rc, st8 = epi_state[b]
                cs = (0, 1) if half == 0 else (2, 3)

                def tr_emit(c):
                    def emit():
                        tp = drp.tile([128, 128], F16, tag="dr", name=f"tr{b}_{c}")
                        nc.tensor.matmul(
                            tp, att[:, c, :, :].rearrange("p h d -> p (h d)"),
                            id_sb, is_transpose=True, start=True, stop=True)
                        t = pe(128, 1)
                        nc.vector.tensor_copy(out=attTs[:, c, :], in_=tp)
                        st8["ev"][c] = dv(128, dep=t, per=0.6)
                    return emit

                def op_emit(c, hf):
                    def emit():
                        op = drp.tile([128, 512], F32, tag="dr",
                                      name=f"op{b}_{c}_{hf}")
                        nc.tensor.matmul(
                            op, attTs[:, c, :],
                            wo_sb[:, hf * 512:(hf + 1) * 512],
                            start=True, stop=True)
                        t = pe(512, 1)
                        if hf == 0:
                            st8["osb"][c] = osbp.tile(
                                [128, HID], F16, tag="osb", name=f"osb{b}_{c}")
                        osb = st8["osb"][c]
                        nc.vector.tensor_copy(
                            out=osb[:, hf * 512:(hf + 1) * 512], in_=op)
                        dv(512, dep=t)
                        if hf == 1:
                            r0 = (4 * b + c) * 128
                            if b <= 1:
                                # sync queue is still draining the 8MB x
                                # prefetch; defer so osb reuse never waits
                                # behind it (in-order DMA ring)
                                deferred_stores.append((r0, osb))
                            else:
                                nc.sync.dma_start(out=out[r0:r0 + 128, :],
                                                  in_=osb)
                    return emit

                for c in cs:
                    epi.append(Unit(lambda half=half: st8["nd"][half] + SEM,
                                    tr_emit(c), blk=b, cost=128 * PE_NS + 50))
                    for hf in range(2):
                        epi.append(Unit(
                            lambda c=c: st8["ev"].get(c, 1e18) + SEM,
                            op_emit(c, hf), blk=b, cost=512 * PE_NS + 20))

            # ---------------- prologue
            push_proj(0)
            push_proj(1)
            filler.pop(0).emit()   # q-proj(0): ring wait covers wq+xt0 only
            nc.sync.dma_start(out=wk_sb.rearrange("p k m -> p (k m)"), in_=wkT[:, :])
            filler.pop(0).emit()   # k-proj(0)
            nc.sync.dma_start(out=wv_sb.rearrange("p k m -> p (k m)"), in_=wvT[:, :])
            filler.pop(0).emit()   # v-proj(0)
            load_xt(1)
            nc.sync.dma_start(out=wo_sb, in_=woT[:, :])

            ot_tiles = {}
            norm_b_pending = [None]
            deferred_stores = []
            bank_free = [0.0, 0.0]
            for b in range(NB):
                if b + 2 < NB:
                    load_xt(b + 2)
                if norm_b_pending[0] is not None:
                    norm_b_pending[0]()
                    norm_b_pending[0] = None
                if b == 2 and deferred_stores:
                    # block-1 op-casts must be emitted (after normB(1)!)
                    # before their deferred stores
                    while any(u.blk <= 1 for u in epi):
                        for i, u in enumerate(epi):
                            if u.blk <= 1:
                                epi.pop(i).emit()
                                break
                    for r0, osb in deferred_stores:
                        nc.sync.dma_start(out=out[r0:r0 + 128, :], in_=osb)
                    deferred_stores.clear()
                # pool-reuse WAR needs older epilogue units emitted before
                # this block's att/attTs tiles are written (in-order queues)
                while any(u.blk <= b - 2 for u in epi):
                    for i, u in enumerate(epi):
                        if u.blk <= b - 2:
                            epi.pop(i).emit()
                            break
                cur_blk[0] = b
                while any(u.blk <= b and u.phase == "pre" for u in filler):
                    for i, u in enumerate(filler):
                        if u.blk <= b and u.phase == "pre":
                            filler.pop(i).emit()
                            break
                if b + 2 < NB:
                    push_proj(b + 2)

                ntl = 4 * (b + 1)
                ots = [otp.tile([128, 4, 65], F32, tag="ot", name=f"ot{b}_{h}")
                       for h in range(HPC)]
                ot_tiles[b] = ots
                make_epi(b)
                ot_done = {"t": 0.0}
                ot_started = [False, False]

                groups = [list(range(g, min(g + GROUP, ntl)))
                          for g in range(0, ntl, GROUP)]
                slots = [(h, grp) for grp in groups for h in range(HPC)]
                pend = []
                si = 0
                for h, grp in slots:
                    hsl = slice(64 * h, 64 * (h + 1))
                    drip_until(bank_free[si % 2])
                    st = stp.tile([128, GROUP * SB], F32, tag="st",
                                  name=f"st{b}_{h}_{grp[0]}")
                    offs = []
                    for i, t in enumerate(grp):
                        off = 128 * (t - 4 * b) if t >= 4 * b else 0
                        offs.append(off)
                        nc.tensor.matmul(
                            st[:, i * SB + off:(i + 1) * SB],
                            kT[hsl, t * 128:(t + 1) * 128],
                            qT[hsl, b * SB + off:(b + 1) * SB],
                            start=True, stop=True)
                    st_done = pe(sum(SB - o for o in offs), len(grp))
                    L = len(grp) * SB
                    pt = ptp.tile([128, GROUP * SB], F16, tag="pt",
                                  name=f"pt{b}_{h}_{grp[0]}")
                    o0 = offs[0]
                    nc.scalar.activation(out=pt[:, o0:L], in_=st[:, o0:L],
                                         func=AF.Exp, scale=float(HD) ** -0.5)
                    exp_done = sc(L - o0, dep=st_done)
                    bank_free[si % 2] = exp_done + SEM
                    mask_done = exp_done
                    for i, t in enumerate(grp):
                        j = t - 4 * b
                        if j >= 0:
                            psl = slice(i * SB + 128 * j, i * SB + 128 * (j + 1))
                            nc.gpsimd.affine_select(
                                out=pt[:, psl], in_=pt[:, psl],
                                pattern=[[1, 128]],
                                compare_op=mybir.AluOpType.is_ge, fill=0.0,
                                channel_multiplier=-1)
                            mask_done = dv(128, dep=exp_done, per=0.6, ov=150.0)

                    def pv_closure(h=h, grp=list(grp), pt=pt, md=mask_done):
                        def emit():
                            drip_until(md + SEM)
                            rows, n = 0, 0
                            for i, t in enumerate(grp):
                                j = t - 4 * b
                                for c in range(max(j, 0), 4):
                                    first = not ot_started[h]
                                    ot_started[h] = True
                                    nc.tensor.matmul(
                                        ots[h][:, c, :],
                                        pt[:, i * SB + 128 * c:
                                           i * SB + 128 * (c + 1)],
                                        vbuf[:, h, t, :],
                                        start=first, stop=(t == 4 * b + c),
                                        skip_group_check=True)
                                    rows += 65
                                    n += 1
                            ot_done["t"] = pe(rows, n)
                        return emit

                    pend.append(pv_closure())
                    if len(pend) > 1:
                        pend.pop(0)()
                    if si == max(len(slots) - 6, 0):
                        while any(u.blk <= b for u in filler):
                            for i, u in enumerate(filler):
                                if u.blk <= b:
                                    filler.pop(i).emit()
                                    break
                    if si == len(slots) - 2:
                        # chains c<=1 of both heads are complete: first-half
                        # epilogue can start while the last slots run
                        norm_half(b, 0, ot_done)()
                        push_tr_op(b, 0)
                    si += 1
                while pend:
                    pend.pop(0)()
                norm_b_pending[0] = norm_half(b, 1, ot_done)
                push_tr_op(b, 1)

            if norm_b_pending[0] is not None:
                norm_b_pending[0]()
                norm_b_pending[0] = None
            while epi or filler:
                if not drip_one():
                    break

    _split_waits(nc)
    return nc


_cached = {}


def _get_nc():
    if "nc" not in _cached:
        _cached["nc"] = _build_nc()
    return _cached["nc"]


def make_in_maps(x, wq, wk, wv, wo):
    x = np.asarray(x, dtype=np.float32)
    wq, wk, wv, wo = (np.asarray(a, dtype=np.float32) for a in (wq, wk, wv, wo))
    B = x.shape[0]
    assert x.shape == (B, S, HID)

    dt = np.float16

    def pack_w(wT):
        # [HID, EPC] -> [128, KH*EPC]: one contiguous 2KB row per partition
        return np.ascontiguousarray(
            wT.reshape(KH, 128, EPC).transpose(1, 0, 2)).reshape(128, KH * EPC)

    xr = x[0].T.astype(dt).reshape(KH, 128, NB, SB)
    xTm = np.ascontiguousarray(xr.transpose(2, 1, 0, 3)).reshape(NB, 128, KH * SB)

    in_maps = []
    for c in range(NCORES):
        esl = slice(c * EPC, (c + 1) * EPC)
        in_maps.append({
            "xT": xTm,
            "wqT": pack_w(wq[esl, :].T.astype(dt)),
            "wkT": pack_w(wk[esl, :].T.astype(dt)),
            "wvT": pack_w(wv[esl, :].T.astype(dt)),
            "woT": np.ascontiguousarray(wo[:, esl].T.astype(dt)),
        })
    return in_maps


def kernel(x, wq, wk, wv, wo):
    B = np.asarray(x).shape[0]
    in_maps = make_in_maps(x, wq, wk, wv, wo)
    nc = _get_nc()
    res = run_bass_kernel_spmd(nc, in_maps, core_ids=list(range(NCORES)))
    acc = res.results[0]["out"].astype(np.float32)
    for c in range(1, NCORES):
        acc = acc + res.results[c]["out"].astype(np.float32)
    return acc.reshape(B, S, HID)


if __name__ == "__main__":
    rng = np.random.default_rng(0)
    x = rng.standard_normal((1, S, HID), dtype=np.float32)
    lim = float(np.sqrt(6.0 / (HID + 16 * HD)))
    wq, wk, wv, wo = (rng.uniform(-lim, lim, (1024, 1024)).astype(np.float32)
                      for _ in range(4))
    got = kernel(x=x, wq=wq, wk=wk, wv=wv, wo=wo)
    print("kernel output", got.shape, got.dtype, got.flat[:4])

